# revision 1
# baseline (speedup 1.0000x reference)
"""AGNN (2-layer) distributed Bass kernel for one TRN2 chip (8 NeuronCores).

Sharding: nodes are row-block-sharded across the 8 cores (6250 each). Each
core owns the edges whose *destination* lands in its node range. Per layer,
each core:
  - builds a 256B node "table" row per local node:
      [xn(64) | rcp_norm | norm | pad(62)] bf16
  - AllGathers the table so every core can gather arbitrary source rows
  - gathers per-edge src rows (from the allgathered table, int16-index
    split into lo/hi halves) and dst rows (from its local table) with the
    dma_gather custom DMA instruction
  - computes per-edge softmax weights w = exp(beta * <xn_src, xn_dst>)
  - scatter-adds via one-hot matmuls on the TensorEngine:
      lhsT = M (one-hot over a 64-dst window, scaled by w * norm_src)
      rhs  = [xn_src | rcp_norm_src]
    which yields per-window [sum_e w*h_src | sum_e w] in PSUM simultaneously
  - divides, renormalizes, and feeds the next layer.
lin1/lin2 GEMMs are node-parallel; log_softmax is fused at the end.

The edge structure (indices, window bucketing, lo/hi split, padding) is
computed on the host from edge_index at call time and passed in as per-core
int16/bf16 data; the compiled program is identical across cores (SPMD).
"""

import contextlib
import numpy as np
import ml_dtypes

from concourse import bacc, bass, mybir, tile

BF16 = mybir.dt.bfloat16
F32 = mybir.dt.float32
I16 = mybir.dt.int16
NPBF = ml_dtypes.bfloat16

EPS = 1e-12
# Tile cost-model end-to-end estimate (measured from the scheduler's
# simulation trace at WAVE_WINDOWS=4; no NTFF profiling is available in this
# container). Engine busy there: DVE 528us, Pool 416us, ACT 140us, PE 116us.
# The shipped WAVE_WINDOWS=8 + psum bufs=1 config halves per-wave fixed
# overheads (full-size HW verified, rel err 1.66e-3). DVE remains the
# bottleneck; next candidates: shrink the M01 mask-build window and reduce
# the 62 pad bytes per 256B gathered row. NOTE: dma_gather hangs the chip
# above ~1024 indices per call (1536 verified to hang, 1024 verified safe);
# SEG=8 is the empirical ring ceiling, do not raise it.
LAST_COST_MODEL_NS = 710_000
ROW = 128          # table row width (bf16 elements) = 256 bytes
XN, RCPN, NORM = 0, 64, 65   # column layout within a table row


class Cfg:
    def __init__(self, N=50000, E=800000, F_in=1024, H=64, C=256, P=8,
                 WIN=64, CHUNK=128, WAVE_WINDOWS=8, SPLIT=32768,
                 debug_max_waves=None, debug_layers=2):
        self.N, self.E, self.F_in, self.H, self.C, self.P = N, E, F_in, H, C, P
        self.WIN, self.CHUNK = WIN, CHUNK
        self.WAVE_WINDOWS = WAVE_WINDOWS
        self.SPLIT = min(SPLIT, N)
        self.debug_max_waves = debug_max_waves
        self.debug_layers = debug_layers
        assert N % P == 0
        self.n_loc = N // P
        self.n_win = (self.n_loc + WIN - 1) // WIN
        self.n_rt = (self.n_loc + 127) // 128
        self.rt_tail = self.n_loc - (self.n_rt - 1) * 128
        assert F_in % 128 == 0
        self.n_kc = F_in // 128
        assert self.n_loc < 32768, "local ids must fit int16"


# ----------------------------------------------------------------------------
# host-side edge prep
# ----------------------------------------------------------------------------
def host_prep(cfg, edge_index):
    src = np.asarray(edge_index[0], dtype=np.int64)
    dst = np.asarray(edge_index[1], dtype=np.int64)
    P, n_loc, WIN, CHUNK, n_win = cfg.P, cfg.n_loc, cfg.WIN, cfg.CHUNK, cfg.n_win
    SPLIT = cfg.SPLIT

    core_of = dst // n_loc
    win_of = (dst % n_loc) // WIN
    hi_of = (src >= SPLIT).astype(np.int64)
    # bucket key: (core, window, hi)
    key = (core_of * n_win + win_of) * 2 + hi_of
    order = np.argsort(key, kind="stable")
    bounds = np.searchsorted(key[order], np.arange(P * n_win * 2 + 1))

    def bucket_len(c, w, h):
        i = (c * n_win + w) * 2 + h
        return bounds[i + 1] - bounds[i]

    def bucket(c, w, h):
        i = (c * n_win + w) * 2 + h
        return order[bounds[i]:bounds[i + 1]]

    K_lo = np.zeros(n_win, dtype=np.int64)
    K_hi = np.zeros(n_win, dtype=np.int64)
    for w in range(n_win):
        mlo = max(bucket_len(c, w, 0) for c in range(P))
        mhi = max(bucket_len(c, w, 1) for c in range(P))
        K_lo[w] = max(1, -(-mlo // CHUNK))
        K_hi[w] = -(-mhi // CHUNK)
    c_total = int((K_lo + K_hi).sum())

    # per-chunk metadata (same for all cores)
    win_of_chunk = np.zeros(c_total, dtype=np.int64)
    start_flag = np.zeros(c_total, dtype=bool)
    stop_flag = np.zeros(c_total, dtype=bool)

    # per-core slot arrays
    idx_src_w = np.zeros((P, 128, c_total * 8), dtype=np.int16)
    idx_dst_w = np.zeros((P, 128, c_total * 8), dtype=np.int16)
    dst_rel = np.full((P, 128, c_total), -1.0, dtype=np.float32)

    # build waves of whole windows
    waves = []
    j0 = 0
    for w0 in range(0, n_win, cfg.WAVE_WINDOWS):
        ws = list(range(w0, min(w0 + cfg.WAVE_WINDOWS, n_win)))
        wc_lo = int(sum(K_lo[w] for w in ws))
        wc_hi = int(sum(K_hi[w] for w in ws))
        Wc = wc_lo + wc_hi
        # chunk order in this wave: lo chunks (by window), then hi chunks
        chunk_map = []   # (window, is_hi) per chunk
        for w in ws:
            chunk_map += [(w, 0)] * int(K_lo[w])
        for w in ws:
            chunk_map += [(w, 1)] * int(K_hi[w])
        for k, (w, h) in enumerate(chunk_map):
            win_of_chunk[j0 + k] = w
        # start = first lo chunk of window; stop = last chunk of window
        lastpos = {}
        for k, (w, h) in enumerate(chunk_map):
            lastpos[w] = k
        seen = set()
        for k, (w, h) in enumerate(chunk_map):
            if w not in seen:
                start_flag[j0 + k] = True
                seen.add(w)
            if lastpos[w] == k:
                stop_flag[j0 + k] = True

        # fill per-core indices
        for c in range(P):
            # edge slots of this wave in chunk order
            ei_parts = [bucket(c, w, 0) for w in ws]
            # lo edges laid into lo chunk region, per window
            col = 0
            slots_src = np.zeros(Wc * 128, dtype=np.int64)  # global src id or 0
            slots_dst = np.zeros(Wc * 128, dtype=np.int64)  # local dst id or 0
            slots_rel = np.full((128, Wc), -1.0, dtype=np.float32)
            for wi, w in enumerate(ws):
                el = ei_parts[wi]
                k = np.arange(len(el))
                pos = (col + k // CHUNK) * 128 + k % CHUNK
                slots_src[pos] = src[el]
                slots_dst[pos] = dst[el] - c * n_loc
                slots_rel[k % CHUNK, col + k // CHUNK] = (
                    dst[el] - c * n_loc) - w * WIN
                col += int(K_lo[w])
            for wi, w in enumerate(ws):
                el = bucket(c, w, 1)
                k = np.arange(len(el))
                pos = (col + k // CHUNK) * 128 + k % CHUNK
                slots_src[pos] = src[el] - SPLIT
                slots_dst[pos] = dst[el] - c * n_loc
                slots_rel[k % CHUNK, col + k // CHUNK] = (
                    dst[el] - c * n_loc) - w * WIN
                col += int(K_hi[w])
            dst_rel[c, :, j0:j0 + Wc] = slots_rel

            # wrapped int16 layouts; the lo and hi src gathers each restart
            # their wrap at their own region
            def wrap_into(dest, base_col, vals):
                n = len(vals)
                i = np.arange(n)
                cols = base_col * 8 + i // 16
                rows = i % 16
                for g in range(8):
                    dest[g * 16 + rows, cols] = vals
            wrap_into(idx_src_w[c], j0, slots_src[:wc_lo * 128].astype(np.int16))
            if wc_hi:
                wrap_into(idx_src_w[c], j0 + wc_lo,
                          slots_src[wc_lo * 128:].astype(np.int16))
            wrap_into(idx_dst_w[c], j0, slots_dst.astype(np.int16))

        waves.append(dict(j0=j0, Wc=Wc, WcLo=wc_lo, WcHi=wc_hi,
                          n_wins=len(ws)))
        j0 += Wc

    wc_max = max(v["Wc"] for v in waves)
    return dict(
        idx_src_w=idx_src_w,
        idx_dst_w=idx_dst_w,
        dst_rel=dst_rel.astype(NPBF),
        waves=waves,
        c_total=c_total,
        wc_max=wc_max,
        win_of_chunk=win_of_chunk,
        start_flag=start_flag,
        stop_flag=stop_flag,
    )


# ----------------------------------------------------------------------------
# device program
# ----------------------------------------------------------------------------
def build_program(cfg, prep):
    P, H, C, F_in = cfg.P, cfg.H, cfg.C, cfg.F_in
    n_loc, n_rt, rt_tail, n_kc = cfg.n_loc, cfg.n_rt, cfg.rt_tail, cfg.n_kc
    WIN, N, SPLIT = cfg.WIN, cfg.N, cfg.SPLIT
    c_total, waves, wc_max = prep["c_total"], prep["waves"], prep["wc_max"]
    win_of_chunk = prep["win_of_chunk"]
    start_flag, stop_flag = prep["start_flag"], prep["stop_flag"]

    nc = bacc.Bacc("TRN2", target_bir_lowering=False)

    xT_ext = nc.declare_dram_parameter("xT", [F_in, n_loc], F32, isOutput=False)
    w1t_ext = nc.declare_dram_parameter("w1t", [F_in, H], F32, isOutput=False)
    b1r_ext = nc.declare_dram_parameter("b1r", [128, H], F32, isOutput=False)
    w2t_ext = nc.declare_dram_parameter("w2t", [H, C], F32, isOutput=False)
    b2r_ext = nc.declare_dram_parameter("b2r", [128, C], F32, isOutput=False)
    beta1_ext = nc.declare_dram_parameter("beta1r", [128, 1], F32, isOutput=False)
    beta2_ext = nc.declare_dram_parameter("beta2r", [128, 1], F32, isOutput=False)
    isrc_ext = nc.declare_dram_parameter(
        "idx_src_w", [128, c_total * 8], I16, isOutput=False)
    idst_ext = nc.declare_dram_parameter(
        "idx_dst_w", [128, c_total * 8], I16, isOutput=False)
    drel_ext = nc.declare_dram_parameter(
        "dst_rel", [128, c_total], BF16, isOutput=False)
    iota_ext = nc.declare_dram_parameter("iota64", [128, WIN], BF16, isOutput=False)
    ident_ext = nc.declare_dram_parameter("ident", [128, 128], BF16, isOutput=False)
    out_ext = nc.declare_dram_parameter("out", [n_loc, C], F32, isOutput=True)

    loc_tab = [nc.dram_tensor(f"loc_tab{i}", [n_loc, ROW], BF16) for i in (0, 1)]
    full_tab = [
        nc.dram_tensor(f"full_tab{i}", [N, ROW], BF16, addr_space="Shared")
        for i in (0, 1)
    ]
    replica = [list(range(P))]

    with contextlib.ExitStack() as es:
        tc = es.enter_context(tile.TileContext(nc))
        const = es.enter_context(tc.tile_pool(name="const", bufs=1))
        sb = es.enter_context(tc.tile_pool(name="sb", bufs=1))

        # ------- constants -------
        w1t_f = const.tile([128, n_kc, H], F32)
        nc.sync.dma_start(
            out=w1t_f[:], in_=w1t_ext[:, :].rearrange("(k p) h -> p k h", p=128)
        )
        w1t_b = const.tile([128, n_kc, H], BF16)
        nc.vector.tensor_copy(w1t_b[:], w1t_f[:])

        b1r = const.tile([128, H], F32)
        nc.sync.dma_start(out=b1r[:], in_=b1r_ext[:])
        b2r = const.tile([128, C], F32)
        nc.sync.dma_start(out=b2r[:], in_=b2r_ext[:])
        b2r_b = const.tile([128, C], BF16)
        nc.vector.tensor_copy(b2r_b[:], b2r[:])

        w2t_f = const.tile([H, C], F32)
        nc.sync.dma_start(out=w2t_f[:], in_=w2t_ext[:])
        w2t_b = const.tile([H, C], BF16)
        nc.vector.tensor_copy(w2t_b[:], w2t_f[:])

        beta = []
        for i, ext in enumerate((beta1_ext, beta2_ext)):
            bt = const.tile([128, 1], F32, name=f"beta{i}")
            nc.sync.dma_start(out=bt[:], in_=ext[:])
            beta.append(bt)

        iota64 = const.tile([128, WIN], BF16)
        nc.sync.dma_start(out=iota64[:], in_=iota_ext[:])
        ident = const.tile([128, 128], BF16)
        nc.sync.dma_start(out=ident[:], in_=ident_ext[:])

        isrc = const.tile([128, c_total * 8], I16)
        nc.sync.dma_start(out=isrc[:], in_=isrc_ext[:])
        idst = const.tile([128, c_total * 8], I16)
        nc.sync.dma_start(out=idst[:], in_=idst_ext[:])
        drel = const.tile([128, c_total], BF16)
        nc.sync.dma_start(out=drel[:], in_=drel_ext[:])

        h_loc = sb.tile([128, n_rt, H], BF16)
        nc.vector.memset(h_loc[:], 0.0)

        # ------- phase A: lin1 + relu -------
        nsq = sb.tile([128, n_rt], F32)
        with tc.tile_pool(name="ph_a", bufs=3) as pa, \
             tc.tile_pool(name="ph_a_ps", bufs=2, space="PSUM") as pa_ps:
            for r in range(n_rt):
                rows = 128 if r < n_rt - 1 else rt_tail
                xt_f = pa.tile([128, n_kc, 128], F32, tag="xt_f")
                xt_b = pa.tile([128, n_kc, 128], BF16, tag="xt_b")
                nc.sync.dma_start(
                    out=xt_f[:, :, :rows],
                    in_=xT_ext[:, r * 128:r * 128 + rows].rearrange(
                        "(k p) n -> p k n", p=128
                    ),
                )
                nc.scalar.copy(xt_b[:, :, :], xt_f[:, :, :])
                hp = pa_ps.tile([128, H], F32, tag="hp")
                for k in range(n_kc):
                    nc.tensor.matmul(
                        hp[:rows, :],
                        lhsT=xt_b[:, k, :rows],
                        rhs=w1t_b[:, k, :],
                        start=(k == 0),
                        stop=(k == n_kc - 1),
                    )
                hb = pa.tile([128, H], F32, tag="hb")
                nc.vector.tensor_tensor(
                    out=hb[:rows, :], in0=hp[:rows, :], in1=b1r[:rows, :],
                    op=mybir.AluOpType.add,
                )
                nc.vector.tensor_scalar(
                    out=h_loc[:rows, r, :], in0=hb[:rows, :],
                    scalar1=0.0, scalar2=None, op0=mybir.AluOpType.max,
                )
            sqa = pa.tile([128, n_rt, H], BF16, tag="sqa")
            nc.vector.tensor_tensor(
                out=sqa[:], in0=h_loc[:], in1=h_loc[:],
                op=mybir.AluOpType.mult,
            )
            nc.vector.tensor_reduce(
                out=nsq[:], in_=sqa[:], axis=mybir.AxisListType.X,
                op=mybir.AluOpType.add,
            )

        # ------- table build + allgather -------
        def build_tables(src_tile, nsq_tile, layer):
            norm = sb.tile([128, n_rt], F32, name=f"norm{layer}")
            rcpn = sb.tile([128, n_rt], F32, name=f"rcpn{layer}")
            nc.scalar.sqrt(norm[:], nsq_tile[:])
            nc.vector.tensor_scalar(
                out=norm[:], in0=norm[:], scalar1=float(EPS), scalar2=None,
                op0=mybir.AluOpType.max,
            )
            nc.vector.reciprocal(rcpn[:], norm[:])
            tabrow = sb.tile([128, n_rt, ROW], BF16, name=f"tabrow{layer}")
            nc.vector.memset(tabrow[:], 0.0)
            for r in range(n_rt):
                nc.vector.tensor_scalar(
                    out=tabrow[:, r, XN:XN + H], in0=src_tile[:, r, :],
                    scalar1=rcpn[:, r:r + 1], scalar2=None,
                    op0=mybir.AluOpType.mult,
                )
            nc.vector.tensor_copy(tabrow[:, :, RCPN], rcpn[:, :])
            nc.vector.tensor_copy(tabrow[:, :, NORM], norm[:, :])
            lt = loc_tab[layer]
            nfull = (n_rt - 1) * 128
            if nfull > 0:
                nc.sync.dma_start(
                    out=lt[0:nfull, :].rearrange("(t p) f -> p t f", p=128),
                    in_=tabrow[:, 0:n_rt - 1, :],
                )
            nc.sync.dma_start(
                out=lt[nfull:n_loc, :], in_=tabrow[0:rt_tail, n_rt - 1, :]
            )
            nc.gpsimd.collective_compute(
                "AllGather",
                mybir.AluOpType.bypass,
                replica_groups=replica,
                ins=[lt[:].opt()],
                outs=[full_tab[layer][:].opt()],
            )

        # ------- one AGNN layer -------
        def agnn_layer(layer, beta_tile, agg_out, nsq_out, work, psum):
            ft = full_tab[layer]
            lt = loc_tab[layer]
            den_r = sb.tile([128, n_rt], F32, name=f"den{layer}")
            agg_raw = sb.tile([128, n_rt, H + 1], BF16, name=f"agg{layer}")
            use_waves = waves if cfg.debug_max_waves is None else waves[:cfg.debug_max_waves]
            for g, wave in enumerate(use_waves):
                Wc, j0 = wave["Wc"], wave["j0"]
                WcLo, WcHi = wave["WcLo"], wave["WcHi"]
                gsrc = work.tile([128, wc_max, ROW], BF16, tag="gsrc")
                gdst = work.tile([128, wc_max, ROW], BF16, tag="gdst")
                SEG = 8  # chunks per dma_gather (1024 idxs; ring-safe)

                def seg_gather(out_tile, src_ap, idx_tile, c0, n_chunks):
                    for s0 in range(0, n_chunks, SEG):
                        sn = min(SEG, n_chunks - s0)
                        nc.gpsimd.dma_gather(
                            out_ap=out_tile[:, c0 + s0:c0 + s0 + sn, :],
                            in_ap=src_ap,
                            idxs_ap=idx_tile[:, (j0 + c0 + s0) * 8:
                                             (j0 + c0 + s0 + sn) * 8],
                            num_idxs=sn * 128, num_idxs_reg=sn * 128,
                            elem_size=ROW,
                        )

                seg_gather(gsrc, ft[0:SPLIT, :], isrc, 0, WcLo)
                if WcHi:
                    seg_gather(gsrc, ft[SPLIT:N, :], isrc, WcLo, WcHi)
                seg_gather(gdst, lt[:, :], idst, 0, Wc)
                # per-edge dot via mult + in-place tree reduction (into gdst)
                nc.vector.tensor_tensor(
                    out=gdst[:, 0:Wc, 0:H], in0=gsrc[:, 0:Wc, XN:XN + H],
                    in1=gdst[:, 0:Wc, XN:XN + H], op=mybir.AluOpType.mult,
                )
                width = H
                while width > 2:
                    half = width // 2
                    nc.vector.tensor_tensor(
                        out=gdst[:, 0:Wc, 0:half],
                        in0=gdst[:, 0:Wc, 0:half],
                        in1=gdst[:, 0:Wc, half:width],
                        op=mybir.AluOpType.add,
                    )
                    width = half
                a_t = work.tile([128, wc_max], BF16, tag="a_t")
                nc.vector.tensor_tensor(
                    out=a_t[:, 0:Wc], in0=gdst[:, 0:Wc, 0],
                    in1=gdst[:, 0:Wc, 1], op=mybir.AluOpType.add,
                )
                w_t = work.tile([128, wc_max], BF16, tag="w_t")
                nc.scalar.activation(
                    out=w_t[:, 0:Wc], in_=a_t[:, 0:Wc],
                    func=mybir.ActivationFunctionType.Exp,
                    scale=beta_tile[:, 0:1],
                )
                wn_t = work.tile([128, wc_max], BF16, tag="wn_t")
                nc.vector.tensor_tensor(
                    out=wn_t[:, 0:Wc], in0=w_t[:, 0:Wc],
                    in1=gsrc[:, 0:Wc, NORM], op=mybir.AluOpType.mult,
                )
                mw = work.tile([128, wc_max, WIN], BF16, tag="mw")
                nc.vector.tensor_tensor(
                    out=mw[:, 0:Wc, :],
                    in0=drel[:, j0:j0 + Wc].to_broadcast([128, Wc, WIN]),
                    in1=iota64[:, None, :].broadcast_to([128, Wc, WIN]),
                    op=mybir.AluOpType.is_equal,
                )
                nc.vector.tensor_tensor(
                    out=mw[:, 0:Wc, :], in0=mw[:, 0:Wc, :],
                    in1=wn_t[:, 0:Wc].to_broadcast([128, Wc, WIN]),
                    op=mybir.AluOpType.mult,
                )
                # scatter matmuls: one psum bank per window in this wave
                w_base = int(win_of_chunk[j0])
                pslots = [
                    psum.tile([WIN, H + 1], F32, tag=f"ps{s}", name=f"ps{s}_{layer}_{g}")
                    for s in range(wave["n_wins"])
                ]
                for k in range(Wc):
                    j = j0 + k
                    s = int(win_of_chunk[j] - w_base)
                    nc.tensor.matmul(
                        pslots[s][:, :],
                        lhsT=mw[:, k, :],
                        rhs=gsrc[:, k, XN:XN + H + 1],
                        start=bool(start_flag[j]),
                        stop=bool(stop_flag[j]),
                    )
                for s in range(wave["n_wins"]):
                    w = w_base + s
                    prow = (w % 2) * WIN
                    nc.scalar.copy(
                        agg_raw[prow:prow + WIN, w // 2, :], pslots[s][:, :]
                    )
                    nc.vector.tensor_copy(
                        den_r[prow:prow + WIN, w // 2:w // 2 + 1],
                        pslots[s][:, H:H + 1],
                    )
            # divide
            nc.vector.tensor_scalar(
                out=den_r[:], in0=den_r[:], scalar1=float(EPS), scalar2=None,
                op0=mybir.AluOpType.max,
            )
            rcpden = sb.tile([128, n_rt], F32, name=f"rcpden{layer}")
            nc.vector.reciprocal(rcpden[:], den_r[:])
            rcpden_b = sb.tile([128, n_rt], BF16, name=f"rcpdenb{layer}")
            nc.vector.tensor_copy(rcpden_b[:], rcpden[:])
            nc.vector.tensor_tensor(
                out=agg_out[:, :, :], in0=agg_raw[:, :, 0:H],
                in1=rcpden_b[:, :].to_broadcast([128, n_rt, H]),
                op=mybir.AluOpType.mult,
            )
            sqscr = work.tile([128, n_rt, H], BF16, tag="sqscr")
            nc.vector.tensor_tensor(
                out=sqscr[:], in0=agg_out[:], in1=agg_out[:],
                op=mybir.AluOpType.mult,
            )
            nc.vector.tensor_reduce(
                out=nsq_out[:], in_=sqscr[:], axis=mybir.AxisListType.X,
                op=mybir.AluOpType.add,
            )

        x1 = sb.tile([128, n_rt, H], BF16)
        nsq1 = sb.tile([128, n_rt], F32)
        x2 = sb.tile([128, n_rt, H], BF16)
        nsq2 = sb.tile([128, n_rt], F32)
        with tc.tile_pool(name="work", bufs=2) as work, \
             tc.tile_pool(name="psum_b", bufs=1, space="PSUM") as psum:
            if cfg.debug_layers >= 1:
                build_tables(h_loc, nsq, layer=0)
                agnn_layer(0, beta[0], x1, nsq1, work, psum)
            else:
                nc.vector.memset(x1[:], 0.0)
                nc.vector.memset(nsq1[:], 1.0)
            if cfg.debug_layers >= 2:
                build_tables(x1, nsq1, layer=1)
                agnn_layer(1, beta[1], x2, nsq2, work, psum)
            else:
                nc.vector.memset(x2[:], 0.0)

        # ------- phase C: lin2 + log_softmax -------
        with tc.tile_pool(name="ph_c", bufs=2) as pc, \
             tc.tile_pool(name="ph_c1", bufs=1) as pc1, \
             tc.tile_pool(name="ph_c_ps", bufs=2, space="PSUM") as pc_ps:
            logits = pc1.tile([128, n_rt, C], BF16)
            for r in range(n_rt):
                tp = pc_ps.tile([H, 128], BF16, tag="tp")
                nc.tensor.transpose(tp[:, :], x2[:, r, :], ident[:])
                x2t = pc.tile([H, 128], BF16, tag="x2t")
                nc.scalar.copy(x2t[:], tp[:])
                lp = pc_ps.tile([128, C], F32, tag="lp")
                nc.tensor.matmul(lp[:], lhsT=x2t[:], rhs=w2t_b[:],
                                 start=True, stop=True)
                nc.vector.scalar_tensor_tensor(
                    out=logits[:, r, :], in0=lp[:], scalar=0.0,
                    in1=b2r_b[:], op0=mybir.AluOpType.add,
                    op1=mybir.AluOpType.add,
                )

            mx = pc1.tile([128, n_rt], F32)
            nc.vector.tensor_reduce(
                out=mx[:], in_=logits[:], axis=mybir.AxisListType.X,
                op=mybir.AluOpType.max,
            )
            mx_b = pc1.tile([128, n_rt], BF16)
            nc.vector.tensor_copy(mx_b[:], mx[:])
            nc.vector.tensor_tensor(
                out=logits[:], in0=logits[:],
                in1=mx_b[:, :].to_broadcast([128, n_rt, C]),
                op=mybir.AluOpType.subtract,
            )
            ex = pc1.tile([128, n_rt, C], BF16)
            nc.scalar.activation(
                out=ex[:], in_=logits[:], func=mybir.ActivationFunctionType.Exp
            )
            sm = pc1.tile([128, n_rt], F32)
            nc.vector.tensor_reduce(
                out=sm[:], in_=ex[:], axis=mybir.AxisListType.X,
                op=mybir.AluOpType.add,
            )
            logz = pc1.tile([128, n_rt], F32)
            nc.scalar.activation(
                out=logz[:], in_=sm[:], func=mybir.ActivationFunctionType.Ln
            )
            logz_b = pc1.tile([128, n_rt], BF16)
            nc.vector.tensor_copy(logz_b[:], logz[:])
            outf = pc1.tile([128, n_rt, C], F32)
            nc.vector.tensor_tensor(
                out=outf[:], in0=logits[:],
                in1=logz_b[:, :].to_broadcast([128, n_rt, C]),
                op=mybir.AluOpType.subtract,
            )
            nfull = (n_rt - 1) * 128
            if nfull > 0:
                nc.sync.dma_start(
                    out=out_ext[0:nfull, :].rearrange("(t p) f -> p t f", p=128),
                    in_=outf[:, 0:n_rt - 1, :],
                )
            nc.sync.dma_start(
                out=out_ext[nfull:n_loc, :], in_=outf[0:rt_tail, n_rt - 1, :]
            )

    return nc


# ----------------------------------------------------------------------------
# host entry point
# ----------------------------------------------------------------------------
def make_in_maps(cfg, prep, inputs):
    P, n_loc, H, C, WIN = cfg.P, cfg.n_loc, cfg.H, cfg.C, cfg.WIN
    x = np.asarray(inputs["x"], dtype=np.float32)
    w1 = np.asarray(inputs["lin1_w"], dtype=np.float32)
    b1 = np.asarray(inputs["lin1_b"], dtype=np.float32)
    w2 = np.asarray(inputs["lin2_w"], dtype=np.float32)
    b2 = np.asarray(inputs["lin2_b"], dtype=np.float32)
    beta1 = np.asarray(inputs["beta1"], dtype=np.float32)
    beta2 = np.asarray(inputs["beta2"], dtype=np.float32)

    w1t = np.ascontiguousarray(w1.T)
    w2t = np.ascontiguousarray(w2.T)
    b1r = np.broadcast_to(b1[None, :], (128, H)).copy()
    b2r = np.broadcast_to(b2[None, :], (128, C)).copy()
    b1r_t = np.broadcast_to(beta1.reshape(1, 1), (128, 1)).copy()
    b2r_t = np.broadcast_to(beta2.reshape(1, 1), (128, 1)).copy()
    iota = np.broadcast_to(
        np.arange(WIN, dtype=np.float32)[None, :], (128, WIN)
    ).astype(NPBF).copy()
    ident = np.eye(128, dtype=np.float32).astype(NPBF)

    in_maps = []
    for c in range(P):
        xs = x[c * n_loc:(c + 1) * n_loc]
        in_maps.append(
            {
                "xT": np.ascontiguousarray(xs.T),
                "w1t": w1t,
                "b1r": b1r,
                "w2t": w2t,
                "b2r": b2r,
                "beta1r": b1r_t,
                "beta2r": b2r_t,
                "idx_src_w": prep["idx_src_w"][c],
                "idx_dst_w": prep["idx_dst_w"][c],
                "dst_rel": np.ascontiguousarray(prep["dst_rel"][c]),
                "iota64": iota,
                "ident": ident,
            }
        )
    return in_maps


def run(inputs, trace=False, tmpdir=None, cfg=None):
    from concourse.bass_utils import run_bass_kernel_spmd

    if cfg is None:
        cfg = Cfg()
    prep = host_prep(cfg, np.asarray(inputs["edge_index"]))
    nc = build_program(cfg, prep)
    nc.finalize()
    in_maps = make_in_maps(cfg, prep, inputs)
    res = run_bass_kernel_spmd(
        nc, in_maps, core_ids=list(range(cfg.P)), trace=trace, tmpdir=tmpdir
    )
    outs = [res.results[i]["out"] for i in range(cfg.P)]
    return np.concatenate(outs, axis=0).astype(np.float32), res


def kernel(**inputs) -> np.ndarray:
    out, _ = run(inputs)
    return out


if __name__ == "__main__":
    pass

